# revision 29
# baseline (speedup 1.0000x reference)
"""Trainium2 Bass kernel for per-sample 2-expert MoE residual MLP.

Reference computation (per sample b, expert e = cond[b]):
    h = relu(Wd[e] @ x_b + bd[e])        # [MID, H*W]
    y = Wu[e] @ h + bu[e] + x_b          # [C, H*W]

Shapes: x [8, 1024, 64, 64] f32, Wd [2, 256, 1024], bd [2, 256],
        Wu [2, 1024, 256], bu [2, 1024], cond [8] int.

Sharding: data-parallel over batch - one sample per NeuronCore (8 cores).
The expert gather (Wd[cond[b]]) happens on host while building each
core's input map.

v9 design (HW-measured: baseline 110.5us, v5 72.1, v7 70.0):
  * I/O 16-bit: x/Wd/y fp16, h/Wu fp8e4m3. GEMM2 runs fp8 DoubleRow
    (one 216ns matmul covers both 128-row k-tiles of MID; its nb pairs
    share weights so the no-FWL LDWEIGHTS cost amortizes). GEMM1 stays
    fp16: an all-DR GEMM1 was measured SLOWER (~380ns/MM spacing -
    DoubleRow disables fast-weight-load and LDWEIGHTS serializes) and
    ACT fp16->fp8 casts cost ~1.9us per 2048 cols. Variable stripe
    widths (v8) also measured slower - the x-in stream paces the front
    half regardless, and extra stripe boundaries add PE gaps.
  * Tile deps are whole-tile: x arrives as two half tiles per stripe,
    wd in three k-group tiles, y in per-mc-pair tiles - so consumers
    unblock as soon as their actual bytes land.
  * Software pipeline: GEMM1(s+1) interleaves into GEMM2(s) between
    drain groups so the PE never idles on psum drains. PSUM: GEMM1 two
    1-bank tiles, GEMM2 a 3-deep pool of 2-bank tiles (drains spill
    into the next GEMM1 window).
  * Drains: mc 2-5 via ACT Identity+bias then DVE 2x fp16 add (those
    idents sit mid-stripe in the ACT queue, before the next stripe's
    relu); the rest via fused DVE (psum+bu)+x. The last stripe
    alternates DVE/ACT per mc and stores per-mc for a shorter tail.
  * 8 warmup matmuls on zeroed scratch spend the PE p-state ramp
    (cold matmuls run 2-3x slow) during the DMA lead-in.
  * Queues: sync ring = wd[k=0] + x stream, scalar ring = remaining
    weights, gpsimd SWDGE = all y stores.
"""

import numpy as np
import ml_dtypes
from contextlib import ExitStack

import concourse.bacc as bacc
import concourse.mybir as mybir
import concourse.tile as tile
from concourse.bass_utils import run_bass_kernel_spmd

# Problem dims (hardcoded per contract).
B = 8
C = 1024
MID = 256
H = 64
W = 64
HW = H * W  # 4096

P = 128              # partitions
NB = 512             # matmul free dim / one fp32 PSUM bank
PASS_W = 1024        # spatial columns per stripe
NBP = PASS_W // NB   # psum banks per [P, PASS_W] fp32 tile
PASS_N = HW // PASS_W
KC = C // P          # 8  k-tiles for GEMM1 / mc-tiles for GEMM2
KM = MID // P        # 2  m-tiles for GEMM1 / k-tiles for GEMM2

F32 = mybir.dt.float32
F16 = mybir.dt.float16
F8 = mybir.dt.float8e4
NPF16 = np.float16
NPF8 = ml_dtypes.float8_e4m3 if hasattr(ml_dtypes, "float8_e4m3") \
    else ml_dtypes.float8_e4m3fn

RELU = mybir.ActivationFunctionType.Relu
IDENT = mybir.ActivationFunctionType.Identity
ADD = mybir.AluOpType.add
DR = mybir.MatmulPerfMode.DoubleRow


def build_nc():
    """Build the per-core Bass program (SPMD: same program on all cores)."""
    nc = bacc.Bacc("TRN2", target_bir_lowering=False, debug=False)

    x_d = nc.dram_tensor("x", [PASS_N, P, KC, PASS_W], F16, kind="ExternalInput")
    wdT_d = nc.dram_tensor("wdT", [P, KC, MID], F16, kind="ExternalInput")
    wuT_d = nc.dram_tensor("wuT", [P, KM, C], F8, kind="ExternalInput")
    bd_d = nc.dram_tensor("bd", [P, KM], F32, kind="ExternalInput")
    bu_d = nc.dram_tensor("bu", [P, KC], F32, kind="ExternalInput")
    y_d = nc.dram_tensor("y", [PASS_N, P, KC, PASS_W], F16, kind="ExternalOutput")

    with tile.TileContext(nc) as tc, ExitStack() as ctx:
        wpool = ctx.enter_context(tc.tile_pool(name="w", bufs=1))
        xcpool = ctx.enter_context(tc.tile_pool(name="xc", bufs=4))
        xpool = ctx.enter_context(tc.tile_pool(name="xp", bufs=6))
        hpool = ctx.enter_context(tc.tile_pool(name="hp", bufs=2))
        ypool = ctx.enter_context(tc.tile_pool(name="yp", bufs=6))
        psh = ctx.enter_context(tc.tile_pool(name="ph", bufs=2, space="PSUM"))
        psy = ctx.enter_context(tc.tile_pool(name="py", bufs=3, space="PSUM"))

        # --- prologue: ~0.6us serial issue cost per dma_start, so loads
        # split across BOTH HWDGE rings: sync carries wd[k=0] + the x
        # stream, scalar the other weights.
        wd0 = wpool.tile([P, 1, MID], F16, tag="wd0")
        nc.sync.dma_start(wd0[:], wdT_d[:, 0:1, :])
        bd_s = wpool.tile([P, KM], F32, tag="bd")
        nc.scalar.dma_start(bd_s[:], bd_d[:])

        xc = []  # stripe-0 x, four independent k-pair chunk tiles

        # chunks 2,3 issue from the idle gpsimd SWDGE queue so their
        # descriptor generation overlaps the sync ring's and x1 starts
        # streaming sooner (y stores don't begin until ~18us).
        for i in range(4):
            t = xcpool.tile([P, 2, PASS_W], F16, tag="xc", name=f"xc{i}")
            eng = nc.sync if i < 2 else nc.gpsimd
            eng.dma_start(t[:], x_d[0, :, 2 * i:2 * i + 2, :])
            xc.append(t)
        wd1 = wpool.tile([P, 3, MID], F16, tag="wd1")
        nc.scalar.dma_start(wd1[:], wdT_d[:, 1:4, :])
        wdR = wpool.tile([P, 4, MID], F16, tag="wdR")
        nc.scalar.dma_start(wdR[:], wdT_d[:, 4:8, :])
        wu_s = wpool.tile([P, KM, C], F8, tag="wu")
        nc.scalar.dma_start(wu_s[:], wuT_d[:])
        bu_s = wpool.tile([P, KC], F32, tag="bu")
        nc.scalar.dma_start(bu_s[:], bu_d[:])

        def wd_ap(k):
            if k == 0:
                return wd0, 0
            if k < 4:
                return wd1, k - 1
            return wdR, k - 4

        def emit_load(s):
            """x stripe as TWO half tiles so GEMM1 k=0-3 can start as
            soon as the first 1MB lands (whole-tile dep granularity)."""
            xta = xpool.tile([P, 4, PASS_W], F16, tag="xt", name=f"xt{s}a")
            nc.sync.dma_start(xta[:], x_d[s, :, 0:4, :])
            xtb = xpool.tile([P, 4, PASS_W], F16, tag="xt", name=f"xt{s}b")
            nc.sync.dma_start(xtb[:], x_d[s, :, 4:8, :])
            return xta, xtb

        xpend = {1: emit_load(1), 2: emit_load(2), 3: emit_load(3)}

        # --- PE warmup on zeroed scratch while the DMAs stream in.
        warm = wpool.tile([P, NB + P], F16, tag="warm")
        nc.vector.memset(warm[:], 0.0)
        wps = psy.tile([P, NBP, NB], F32, tag="py", name="warmps")
        for i in range(12):
            nc.tensor.matmul(
                wps[:, 0, :], warm[:, NB:], warm[:, :NB], start=True, stop=True,
            )

        hts = {}

        def gemm1(s, m):
            """GEMM1 m-phase of stripe s (fp16): 16 matmuls + 2 relus."""
            ht = hts[s]
            ph = [psh.tile([P, NB], F32, tag="ph", name=f"ph{s}_{m}{nb}")
                  for nb in range(NBP)]
            for k in range(KC):
                wt, ki = wd_ap(k)
                xh = xpend[s][k // 4]
                for nb in range(NBP):
                    nc.tensor.matmul(
                        ph[nb][:],
                        wt[:, ki, m * P:(m + 1) * P],
                        xh[:, k % 4, nb * NB:(nb + 1) * NB],
                        start=(k == 0),
                        stop=(k == KC - 1),
                    )
            for nb in range(NBP):
                nc.scalar.activation(
                    ht[:, m, nb * NB:(nb + 1) * NB], ph[nb][:],
                    RELU, bias=bd_s[:, m:m + 1])

        def gemm1_s0():
            """Stripe 0, k-outer: matmuls chase the arriving x0 chunks.
            m=0 uses the psh tiles, m=1 borrows a GEMM2-pool psum tile."""
            hts[0] = hpool.tile([P, KM, PASS_W], F8, tag="ht", name="ht0")
            ht = hts[0]
            ph0 = [psh.tile([P, NB], F32, tag="ph", name=f"ph0_{nb}")
                   for nb in range(NBP)]
            ph1 = psy.tile([P, NBP, NB], F32, tag="py", name="ph1")
            for k in range(KC):
                wt, ki = wd_ap(k)
                for m in range(KM):
                    for nb in range(NBP):
                        out = ph0[nb][:] if m == 0 else ph1[:, nb, :]
                        nc.tensor.matmul(
                            out,
                            wt[:, ki, m * P:(m + 1) * P],
                            xc[k // 2][:, k % 2, nb * NB:(nb + 1) * NB],
                            start=(k == 0),
                            stop=(k == KC - 1),
                        )
            for nb in range(NBP):
                nc.scalar.activation(
                    ht[:, 0, nb * NB:(nb + 1) * NB], ph0[nb][:],
                    RELU, bias=bd_s[:, 0:1])
            for nb in range(NBP):
                nc.scalar.activation(
                    ht[:, 1, nb * NB:(nb + 1) * NB], ph1[:, nb, :],
                    RELU, bias=bd_s[:, 1:2])

        yts = {}

        def xres(s, mc):
            if s == 0:
                return xc[mc // 2][:, mc % 2, :]
            return xpend[s][mc // 4][:, mc % 4, :]

        def gemm2(s, mcs):
            """GEMM2 (fp8 DoubleRow) + drains for the given mc's.
            y-DMAs ride the gpsimd SWDGE queue (per pair; per mc on the
            last stripe so the final store leaves sooner)."""
            ht = hts[s]
            # ACT-drained mc's must be emitted BEFORE the next stripe's
            # relu in the ACT queue (mc 3,4 sit mid-stripe). The last
            # stripe pushes 6 of 8 drains through the otherwise-idle ACT
            # so the two serial drain chains finish together (~6.4us).
            act_drain = (lambda mc: mc not in (0, 3)) if s == PASS_N - 1 \
                else (lambda mc: mc in (3, 4))
            for mc in mcs:
                q, j = divmod(mc, 2)
                if j == 0:
                    yts[s, q] = ypool.tile([P, 2, PASS_W], F16, tag="yt",
                                           name=f"yt{s}_{q}")
                yt = yts[s, q]
                py = psy.tile([P, NBP, NB], F32, tag="py", name=f"py{s}_{mc}")
                for nb in range(NBP):
                    nc.tensor.matmul(
                        py[:, nb, :],
                        wu_s[:, :, mc * P:(mc + 1) * P],
                        ht[:, :, nb * NB:(nb + 1) * NB],
                        start=True,
                        stop=True,
                        perf_mode=DR,
                    )
                if act_drain(mc):
                    nc.scalar.activation(
                        yt[:, j, :], py[:], IDENT, bias=bu_s[:, mc:mc + 1])
                    # all-fp16 add -> DVE 2x packed mode (~0.65us)
                    nc.vector.tensor_tensor(
                        yt[:, j, :], yt[:, j, :], xres(s, mc), ADD)
                else:
                    nc.vector.scalar_tensor_tensor(
                        yt[:, j, :], py[:], bu_s[:, mc:mc + 1], xres(s, mc),
                        ADD, ADD)
                if s == PASS_N - 1:
                    # per-mc stores on alternating queues: the final
                    # 0.25MB leaves right after its own drain.
                    eng = nc.gpsimd if mc % 2 == 0 else nc.scalar
                    eng.dma_start(
                        y_d[s, :, mc:mc + 1, :], yt[:, j:j + 1, :])
                elif j == 1:
                    nc.gpsimd.dma_start(
                        y_d[s, :, 2 * q:2 * q + 2, :], yt[:])

        # Software pipeline: GEMM1 of stripe s+1 interleaves into GEMM2
        # of stripe s so the PE never idles on psum drains. Stripe 0's
        # GEMM2 runs longer before GEMM1(1) (x1 still streaming in).
        gemm1_s0()
        hts[1] = hpool.tile([P, KM, PASS_W], F8, tag="ht", name="ht1")
        gemm2(0, [0, 1, 2, 3, 4])
        gemm1(1, 0)
        gemm2(0, [5, 6, 7])
        gemm1(1, 1)
        for s in (1, 2):
            hts[s + 1] = hpool.tile([P, KM, PASS_W], F8, tag="ht",
                                    name=f"ht{s + 1}")
            gemm2(s, [0, 1])
            gemm1(s + 1, 0)
            gemm2(s, [2, 3, 4])
            gemm1(s + 1, 1)
            gemm2(s, [5, 6, 7])
        gemm2(3, [0, 1, 2, 3, 4, 5, 6, 7])

    nc.compile()
    return nc


_NC = None


def get_nc():
    global _NC
    if _NC is None:
        _NC = build_nc()
    return _NC


def make_in_maps(inputs):
    x = np.asarray(inputs["x"], dtype=np.float32)
    Wd = np.asarray(inputs["Wd"], dtype=np.float32)
    bd = np.asarray(inputs["bd"], dtype=np.float32)
    Wu = np.asarray(inputs["Wu"], dtype=np.float32)
    bu = np.asarray(inputs["bu"], dtype=np.float32)
    cond = np.asarray(inputs["cond"]).astype(np.int64)

    # [C, HW] -> stripe-major [S, P, KC, W] (c = kc*P + p, col = s*PASS_W + w)
    xs = x.reshape(B, KC, P, PASS_N, PASS_W).transpose(0, 3, 2, 1, 4)
    xs = np.ascontiguousarray(xs).astype(NPF16)

    # Per-expert pre-tiled weights (2 experts only -> build once, index).
    wdT = {}
    wuT = {}
    bdT = {}
    buT = {}
    for e in range(2):
        wdT[e] = np.ascontiguousarray(
            Wd[e].T.reshape(KC, P, MID).transpose(1, 0, 2)).astype(NPF16)
        wuT[e] = np.ascontiguousarray(
            Wu[e].T.reshape(KM, P, C).transpose(1, 0, 2)).astype(NPF8)
        bdT[e] = np.ascontiguousarray(bd[e].reshape(KM, P).T)
        buT[e] = np.ascontiguousarray(bu[e].reshape(KC, P).T)

    in_maps = []
    for b in range(B):
        e = int(cond[b])
        in_maps.append({
            "x": xs[b],
            "wdT": wdT[e],
            "wuT": wuT[e],
            "bd": bdT[e],
            "bu": buT[e],
        })
    return in_maps


def run_sharded(inputs, **kwargs):
    """Run on all 8 cores; returns (stacked output [B,C,H,W], results)."""
    nc = get_nc()
    in_maps = make_in_maps(inputs)
    res = run_bass_kernel_spmd(nc, in_maps, core_ids=list(range(B)), **kwargs)
    out = np.empty((B, C, H, W), dtype=np.float32)
    for b in range(B):
        yb = np.asarray(res.results[b]["y"])  # [S, P, KC, W] fp16
        out[b] = yb.transpose(2, 1, 0, 3).reshape(C, HW).astype(np.float32) \
            .reshape(C, H, W)
    return out, res


def kernel(**inputs) -> np.ndarray:
    out, _ = run_sharded(inputs)
    return out


# revision 32
# speedup vs baseline: 1.0576x; 1.0576x over previous
"""Trainium2 Bass kernel for per-sample 2-expert MoE residual MLP.

Reference computation (per sample b, expert e = cond[b]):
    h = relu(Wd[e] @ x_b + bd[e])        # [MID, H*W]
    y = Wu[e] @ h + bu[e] + x_b          # [C, H*W]

Shapes: x [8, 1024, 64, 64] f32, Wd [2, 256, 1024], bd [2, 256],
        Wu [2, 1024, 256], bu [2, 1024], cond [8] int.

Sharding: data-parallel over batch - one sample per NeuronCore (8 cores).
The expert gather (Wd[cond[b]]) happens on host while building each
core's input map.

v9 design (HW-measured: baseline 110.5us, v5 72.1, v7 70.0):
  * I/O 16-bit: x/Wd/y fp16, h/Wu fp8e4m3. GEMM2 runs fp8 DoubleRow
    (one 216ns matmul covers both 128-row k-tiles of MID; its nb pairs
    share weights so the no-FWL LDWEIGHTS cost amortizes). GEMM1 stays
    fp16: an all-DR GEMM1 was measured SLOWER (~380ns/MM spacing -
    DoubleRow disables fast-weight-load and LDWEIGHTS serializes) and
    ACT fp16->fp8 casts cost ~1.9us per 2048 cols. Variable stripe
    widths (v8) also measured slower - the x-in stream paces the front
    half regardless, and extra stripe boundaries add PE gaps.
  * Tile deps are whole-tile: x arrives as two half tiles per stripe,
    wd in three k-group tiles, y in per-mc-pair tiles - so consumers
    unblock as soon as their actual bytes land.
  * Software pipeline: GEMM1(s+1) interleaves into GEMM2(s) between
    drain groups so the PE never idles on psum drains. PSUM: GEMM1 two
    1-bank tiles, GEMM2 a 3-deep pool of 2-bank tiles (drains spill
    into the next GEMM1 window).
  * Drains: mc 2-5 via ACT Identity+bias then DVE 2x fp16 add (those
    idents sit mid-stripe in the ACT queue, before the next stripe's
    relu); the rest via fused DVE (psum+bu)+x. The last stripe
    alternates DVE/ACT per mc and stores per-mc for a shorter tail.
  * 8 warmup matmuls on zeroed scratch spend the PE p-state ramp
    (cold matmuls run 2-3x slow) during the DMA lead-in.
  * Queues: sync ring = wd[k=0] + x stream, scalar ring = remaining
    weights, gpsimd SWDGE = all y stores.
"""

import numpy as np
import ml_dtypes
from contextlib import ExitStack

import concourse.bacc as bacc
import concourse.mybir as mybir
import concourse.tile as tile
from concourse.bass_utils import run_bass_kernel_spmd

# Problem dims (hardcoded per contract).
B = 8
C = 1024
MID = 256
H = 64
W = 64
HW = H * W  # 4096

P = 128              # partitions
NB = 512             # matmul free dim / one fp32 PSUM bank
PASS_W = 1024        # spatial columns per stripe
NBP = PASS_W // NB   # psum banks per [P, PASS_W] fp32 tile
PASS_N = HW // PASS_W
KC = C // P          # 8  k-tiles for GEMM1 / mc-tiles for GEMM2
KM = MID // P        # 2  m-tiles for GEMM1 / k-tiles for GEMM2

F32 = mybir.dt.float32
F16 = mybir.dt.float16
F8 = mybir.dt.float8e4
NPF16 = np.float16
NPF8 = ml_dtypes.float8_e4m3 if hasattr(ml_dtypes, "float8_e4m3") \
    else ml_dtypes.float8_e4m3fn

RELU = mybir.ActivationFunctionType.Relu
IDENT = mybir.ActivationFunctionType.Identity
ADD = mybir.AluOpType.add
DR = mybir.MatmulPerfMode.DoubleRow


def build_nc():
    """Build the per-core Bass program (SPMD: same program on all cores)."""
    nc = bacc.Bacc("TRN2", target_bir_lowering=False, debug=False)

    x_d = nc.dram_tensor("x", [PASS_N, P, KC, PASS_W], F16, kind="ExternalInput")
    wdT_d = nc.dram_tensor("wdT", [P, KC, MID], F16, kind="ExternalInput")
    wuT_d = nc.dram_tensor("wuT", [P, KM, C], F8, kind="ExternalInput")
    bd_d = nc.dram_tensor("bd", [P, KM], F32, kind="ExternalInput")
    bu_d = nc.dram_tensor("bu", [P, KC], F32, kind="ExternalInput")
    y_d = nc.dram_tensor("y", [PASS_N, P, KC, PASS_W], F16, kind="ExternalOutput")

    with tile.TileContext(nc) as tc, ExitStack() as ctx:
        wpool = ctx.enter_context(tc.tile_pool(name="w", bufs=1))
        xcpool = ctx.enter_context(tc.tile_pool(name="xc", bufs=4))
        xpool = ctx.enter_context(tc.tile_pool(name="xp", bufs=6))
        hpool = ctx.enter_context(tc.tile_pool(name="hp", bufs=2))
        ypool = ctx.enter_context(tc.tile_pool(name="yp", bufs=6))
        psh = ctx.enter_context(tc.tile_pool(name="ph", bufs=2, space="PSUM"))
        psy = ctx.enter_context(tc.tile_pool(name="py", bufs=3, space="PSUM"))

        # --- prologue: ~0.6us serial issue cost per dma_start, so loads
        # split across BOTH HWDGE rings: sync carries wd[k=0] + the x
        # stream, scalar the other weights.
        wd0 = wpool.tile([P, 1, MID], F16, tag="wd0")
        nc.sync.dma_start(wd0[:], wdT_d[:, 0:1, :])
        bd_s = wpool.tile([P, KM], F32, tag="bd")
        nc.scalar.dma_start(bd_s[:], bd_d[:])

        xc = []  # stripe-0 x, four independent k-pair chunk tiles

        for i in range(4):
            t = xcpool.tile([P, 2, PASS_W], F16, tag="xc", name=f"xc{i}")
            nc.sync.dma_start(t[:], x_d[0, :, 2 * i:2 * i + 2, :])
            xc.append(t)
        wd1 = wpool.tile([P, 3, MID], F16, tag="wd1")
        nc.scalar.dma_start(wd1[:], wdT_d[:, 1:4, :])
        wdR = wpool.tile([P, 4, MID], F16, tag="wdR")
        nc.scalar.dma_start(wdR[:], wdT_d[:, 4:8, :])
        wu_s = wpool.tile([P, KM, C], F8, tag="wu")
        nc.scalar.dma_start(wu_s[:], wuT_d[:])
        bu_s = wpool.tile([P, KC], F32, tag="bu")
        nc.scalar.dma_start(bu_s[:], bu_d[:])

        def wd_ap(k):
            if k == 0:
                return wd0, 0
            if k < 4:
                return wd1, k - 1
            return wdR, k - 4

        def emit_load(s):
            """x stripe as TWO half tiles so GEMM1 k=0-3 can start as
            soon as the first 1MB lands (whole-tile dep granularity)."""
            xta = xpool.tile([P, 4, PASS_W], F16, tag="xt", name=f"xt{s}a")
            nc.sync.dma_start(xta[:], x_d[s, :, 0:4, :])
            xtb = xpool.tile([P, 4, PASS_W], F16, tag="xt", name=f"xt{s}b")
            nc.sync.dma_start(xtb[:], x_d[s, :, 4:8, :])
            return xta, xtb

        xpend = {1: emit_load(1), 2: emit_load(2), 3: emit_load(3)}

        # --- PE warmup on zeroed scratch while the DMAs stream in.
        warm = wpool.tile([P, NB + P], F16, tag="warm")
        nc.vector.memset(warm[:], 0.0)
        wps = psy.tile([P, NBP, NB], F32, tag="py", name="warmps")
        for i in range(12):
            nc.tensor.matmul(
                wps[:, 0, :], warm[:, NB:], warm[:, :NB], start=True, stop=True,
            )

        hts = {}

        def gemm1(s, m):
            """GEMM1 m-phase of stripe s (fp16): 16 matmuls + 2 relus."""
            ht = hts[s]
            ph = [psh.tile([P, NB], F32, tag="ph", name=f"ph{s}_{m}{nb}")
                  for nb in range(NBP)]
            for k in range(KC):
                wt, ki = wd_ap(k)
                xh = xpend[s][k // 4]
                for nb in range(NBP):
                    nc.tensor.matmul(
                        ph[nb][:],
                        wt[:, ki, m * P:(m + 1) * P],
                        xh[:, k % 4, nb * NB:(nb + 1) * NB],
                        start=(k == 0),
                        stop=(k == KC - 1),
                    )
            for nb in range(NBP):
                nc.scalar.activation(
                    ht[:, m, nb * NB:(nb + 1) * NB], ph[nb][:],
                    RELU, bias=bd_s[:, m:m + 1])

        def gemm1_s0():
            """Stripe 0, k-outer: matmuls chase the arriving x0 chunks.
            m=0 uses the psh tiles, m=1 borrows a GEMM2-pool psum tile."""
            hts[0] = hpool.tile([P, KM, PASS_W], F8, tag="ht", name="ht0")
            ht = hts[0]
            ph0 = [psh.tile([P, NB], F32, tag="ph", name=f"ph0_{nb}")
                   for nb in range(NBP)]
            ph1 = psy.tile([P, NBP, NB], F32, tag="py", name="ph1")
            for k in range(KC):
                wt, ki = wd_ap(k)
                for m in range(KM):
                    for nb in range(NBP):
                        out = ph0[nb][:] if m == 0 else ph1[:, nb, :]
                        nc.tensor.matmul(
                            out,
                            wt[:, ki, m * P:(m + 1) * P],
                            xc[k // 2][:, k % 2, nb * NB:(nb + 1) * NB],
                            start=(k == 0),
                            stop=(k == KC - 1),
                        )
            for nb in range(NBP):
                nc.scalar.activation(
                    ht[:, 0, nb * NB:(nb + 1) * NB], ph0[nb][:],
                    RELU, bias=bd_s[:, 0:1])
            for nb in range(NBP):
                nc.scalar.activation(
                    ht[:, 1, nb * NB:(nb + 1) * NB], ph1[:, nb, :],
                    RELU, bias=bd_s[:, 1:2])

        yts = {}

        def xres(s, mc):
            if s == 0:
                return xc[mc // 2][:, mc % 2, :]
            return xpend[s][mc // 4][:, mc % 4, :]

        def gemm2(s, mcs):
            """GEMM2 (fp8 DoubleRow) + drains for the given mc's.
            y-DMAs ride the gpsimd SWDGE queue (per pair; per mc on the
            last stripe so the final store leaves sooner)."""
            ht = hts[s]
            # ACT-drained mc's must be emitted BEFORE the next stripe's
            # relu in the ACT queue (mc 3,4 sit mid-stripe); the last
            # stripe alternates engines per mc.
            act_drain = (lambda mc: mc % 2 == 1) if s == PASS_N - 1 \
                else (lambda mc: mc in (3, 4))
            for mc in mcs:
                q, j = divmod(mc, 2)
                if j == 0:
                    yts[s, q] = ypool.tile([P, 2, PASS_W], F16, tag="yt",
                                           name=f"yt{s}_{q}")
                yt = yts[s, q]
                py = psy.tile([P, NBP, NB], F32, tag="py", name=f"py{s}_{mc}")
                for nb in range(NBP):
                    nc.tensor.matmul(
                        py[:, nb, :],
                        wu_s[:, :, mc * P:(mc + 1) * P],
                        ht[:, :, nb * NB:(nb + 1) * NB],
                        start=True,
                        stop=True,
                        perf_mode=DR,
                    )
                if act_drain(mc):
                    nc.scalar.activation(
                        yt[:, j, :], py[:], IDENT, bias=bu_s[:, mc:mc + 1])
                    # all-fp16 add -> DVE 2x packed mode (~0.65us)
                    nc.vector.tensor_tensor(
                        yt[:, j, :], yt[:, j, :], xres(s, mc), ADD)
                else:
                    nc.vector.scalar_tensor_tensor(
                        yt[:, j, :], py[:], bu_s[:, mc:mc + 1], xres(s, mc),
                        ADD, ADD)
                if s == PASS_N - 1:
                    nc.gpsimd.dma_start(
                        y_d[s, :, mc:mc + 1, :], yt[:, j:j + 1, :])
                elif j == 1:
                    nc.gpsimd.dma_start(
                        y_d[s, :, 2 * q:2 * q + 2, :], yt[:])

        # Software pipeline: GEMM1 of stripe s+1 interleaves into GEMM2
        # of stripe s so the PE never idles on psum drains. Stripe 0's
        # GEMM2 runs longer before GEMM1(1) (x1 still streaming in).
        gemm1_s0()
        hts[1] = hpool.tile([P, KM, PASS_W], F8, tag="ht", name="ht1")
        gemm2(0, [0, 1, 2, 3, 4])
        gemm1(1, 0)
        gemm2(0, [5, 6, 7])
        gemm1(1, 1)
        for s in (1, 2):
            hts[s + 1] = hpool.tile([P, KM, PASS_W], F8, tag="ht",
                                    name=f"ht{s + 1}")
            gemm2(s, [0, 1])
            gemm1(s + 1, 0)
            gemm2(s, [2, 3, 4])
            gemm1(s + 1, 1)
            gemm2(s, [5, 6, 7])
        gemm2(3, [0, 1, 2, 3, 4, 5, 6, 7])

    nc.compile()
    return nc


_NC = None


def get_nc():
    global _NC
    if _NC is None:
        _NC = build_nc()
    return _NC


def make_in_maps(inputs):
    x = np.asarray(inputs["x"], dtype=np.float32)
    Wd = np.asarray(inputs["Wd"], dtype=np.float32)
    bd = np.asarray(inputs["bd"], dtype=np.float32)
    Wu = np.asarray(inputs["Wu"], dtype=np.float32)
    bu = np.asarray(inputs["bu"], dtype=np.float32)
    cond = np.asarray(inputs["cond"]).astype(np.int64)

    # [C, HW] -> stripe-major [S, P, KC, W] (c = kc*P + p, col = s*PASS_W + w)
    xs = x.reshape(B, KC, P, PASS_N, PASS_W).transpose(0, 3, 2, 1, 4)
    xs = np.ascontiguousarray(xs).astype(NPF16)

    # Per-expert pre-tiled weights (2 experts only -> build once, index).
    wdT = {}
    wuT = {}
    bdT = {}
    buT = {}
    for e in range(2):
        wdT[e] = np.ascontiguousarray(
            Wd[e].T.reshape(KC, P, MID).transpose(1, 0, 2)).astype(NPF16)
        wuT[e] = np.ascontiguousarray(
            Wu[e].T.reshape(KM, P, C).transpose(1, 0, 2)).astype(NPF8)
        bdT[e] = np.ascontiguousarray(bd[e].reshape(KM, P).T)
        buT[e] = np.ascontiguousarray(bu[e].reshape(KC, P).T)

    in_maps = []
    for b in range(B):
        e = int(cond[b])
        in_maps.append({
            "x": xs[b],
            "wdT": wdT[e],
            "wuT": wuT[e],
            "bd": bdT[e],
            "bu": buT[e],
        })
    return in_maps


def run_sharded(inputs, **kwargs):
    """Run on all 8 cores; returns (stacked output [B,C,H,W], results)."""
    nc = get_nc()
    in_maps = make_in_maps(inputs)
    res = run_bass_kernel_spmd(nc, in_maps, core_ids=list(range(B)), **kwargs)
    out = np.empty((B, C, H, W), dtype=np.float32)
    for b in range(B):
        yb = np.asarray(res.results[b]["y"])  # [S, P, KC, W] fp16
        out[b] = yb.transpose(2, 1, 0, 3).reshape(C, HW).astype(np.float32) \
            .reshape(C, H, W)
    return out, res


def kernel(**inputs) -> np.ndarray:
    out, _ = run_sharded(inputs)
    return out
